# revision 1
# baseline (speedup 1.0000x reference)
"""Distributed Trainium2 Bass kernel for a single attention head.

Problem (hardcoded): q,k,v [4, 4096, 1024] f32, Wq/Wk/Wv [1024, 64] f32,
attn_mask [4096, 4096] bool (True = keep).  out[b] = softmax(mask(q Wq (k Wk)^T) / 8) (v Wv).

Sharding: 8 cores; core c -> batch c//2, and query chunks {2s + c%2 : s in 0..3}
(512 rows each, interleaved so the causal-mask work per program slot is uniform
across cores -- all cores execute one SPMD program).

Host-side prep is layout only: transposes to [d, t], weight concat, and
mask block analysis (all-keep / any-keep per 512x128 block) which drives a
compile-time skip schedule for fully-masked score tiles.  Matmuls run in
float32r (verifier requires operands produced as f32r); the mask is applied
additively (-1e30) on the score PSUM before the exp.
"""

import os
import sys

sys.path.insert(0, "/opt/trn_rl_repo")

import numpy as np

import concourse.bass as bass
import concourse.mybir as mybir
import concourse.tile as tile
from concourse import bacc
from concourse.bass_utils import run_bass_kernel_spmd
from concourse.masks import make_identity

F32 = mybir.dt.float32
F32R = mybir.dt.float32r

N_CORES = 8
B, T, D, H = 4, 4096, 1024, 64
P = 128                      # partitions
QC = 512                     # query chunk width
N_CHUNKS = T // QC           # 8 global query chunks
N_SLOTS = N_CHUNKS // 2      # 4 chunks per core
KT = T // P                  # 32 k-tiles of 128 rows
D_TILES = D // P             # 8
XCW = 1024                   # x-chunk width for streaming projections
TQ = N_SLOTS * QC            # 2048 local query rows per core
NEG = -1.0e30                # additive mask value for dropped positions

LAST_RESULT = None           # test harness reads exec_time_ns from here
_CACHE = {}


def _mask_schedule(mask):
    """extents[s]: #k-tiles to process for slot s; need[s][t]: mask add
    needed.  Must be valid for BOTH chunks {2s, 2s+1} (the two cores' views)."""
    m = mask.reshape(N_CHUNKS, QC, KT, P)
    blk_any = m.any(axis=(1, 3))   # [chunk, ktile]
    blk_all = m.all(axis=(1, 3))
    extents = []
    need = []
    for s in range(N_SLOTS):
        js = (2 * s, 2 * s + 1)
        ext = 1
        for j in js:
            nz = np.nonzero(blk_any[j])[0]
            if len(nz):
                ext = max(ext, int(nz[-1]) + 1)
        extents.append(ext)
        need.append([bool((~blk_all[js, t]).any()) for t in range(ext)])
    return tuple(extents), tuple(tuple(n) for n in need)


def _mask_order(extents, need):
    """(s, t) pairs needing a mask tile, in program emission order (t-outer)."""
    order = []
    for ch in range((max(extents) + XCW // P - 1) // (XCW // P)):
        for t in range(ch * (XCW // P), (ch + 1) * (XCW // P)):
            for s in range(N_SLOTS):
                if t < extents[s] and need[s][t]:
                    order.append((s, t))
    return order


def _build(extents, need):
    n_mask = max(1, len(_mask_order(extents, need)))
    nc = bacc.Bacc("TRN2", target_bir_lowering=False, debug=False,
                   num_devices=N_CORES)
    qT = nc.dram_tensor("qT", [D, TQ], F32R, kind="ExternalInput")
    kT = nc.dram_tensor("kT", [D, T], F32R, kind="ExternalInput")
    vT = nc.dram_tensor("vT", [D, T], F32R, kind="ExternalInput")
    w = nc.dram_tensor("w", [D, 5 * H], F32R, kind="ExternalInput")
    maskp = nc.dram_tensor("maskp", [n_mask, P, QC], F32, kind="ExternalInput")
    out = nc.dram_tensor("out", [TQ, H], F32, kind="ExternalOutput")

    Exp = mybir.ActivationFunctionType.Exp
    n_kv_chunks = (max(extents) * P + XCW - 1) // XCW  # k/v chunks actually needed
    kt_lim = max(extents)

    with tile.TileContext(nc) as tc:
        with (
            tc.tile_pool(name="const", bufs=1) as cpool,
            tc.tile_pool(name="qkh", bufs=1) as qkhpool,
            tc.tile_pool(name="vh", bufs=1) as vhpool,
            tc.tile_pool(name="oacc", bufs=1, space="PSUM") as opool,
        ):
            w_sb = cpool.tile([P, D_TILES, 5 * H], F32R)
            nc.sync.dma_start(
                out=w_sb[:], in_=w.ap().rearrange("(dt p) n -> p dt n", p=P))
            ident = cpool.tile([P, P], F32)
            make_identity(nc, ident[:])

            qhT = qkhpool.tile([P, TQ], F32R, tag="qhT")
            khT = qkhpool.tile([P, T], F32R, tag="khT")
            vh1 = vhpool.tile([P, KT, H + 1], F32R)

            oaccs = [opool.tile([H + 1, QC], F32, tag=f"oacc{s}",
                                name=f"oacc{s}")
                     for s in range(N_SLOTS)]

            with (
                tc.tile_pool(name="xs", bufs=2) as xpool,
                tc.tile_pool(name="pps", bufs=2, space="PSUM") as pppool,
                tc.tile_pool(name="sps", bufs=2, space="PSUM") as spool,
                tc.tile_pool(name="pt", bufs=3) as ppool,
                tc.tile_pool(name="mt", bufs=3) as mpool,
                tc.tile_pool(name="vtmp", bufs=2) as vtpool,
            ):
                # ---- phase 0: project q -> qhT [64, TQ] ----
                for ch in range(TQ // XCW):
                    xt = xpool.tile([P, D_TILES, XCW], F32R, tag="x")
                    nc.sync.dma_start(
                        out=xt[:],
                        in_=qT[:, ch * XCW:(ch + 1) * XCW].rearrange(
                            "(dt p) t -> p dt t", p=P))
                    for n in range(XCW // QC):
                        ps = pppool.tile([P, QC], F32, tag="pp")
                        for dt_ in range(D_TILES):
                            nc.tensor.matmul(
                                ps[:],
                                lhsT=w_sb[:, dt_, 0:P],
                                rhs=xt[:, dt_, n * QC:(n + 1) * QC],
                                start=(dt_ == 0), stop=(dt_ == D_TILES - 1))
                        col = ch * XCW + n * QC
                        nc.scalar.copy(out=qhT[:, col:col + QC], in_=ps[:])

                # ---- phase 1: stream k/v chunks; project; attention tiles ----
                mask_idx = 0
                for ch in range(n_kv_chunks):
                    for src, which in ((kT, "k"), (vT, "v")):
                        xt = xpool.tile([P, D_TILES, XCW], F32R, tag="x")
                        nc.sync.dma_start(
                            out=xt[:],
                            in_=src[:, ch * XCW:(ch + 1) * XCW].rearrange(
                                "(dt p) t -> p dt t", p=P))
                        for n in range(XCW // QC):
                            col = ch * XCW + n * QC
                            if which == "k":
                                ps = pppool.tile([P, QC], F32, tag="pp")
                                for dt_ in range(D_TILES):
                                    nc.tensor.matmul(
                                        ps[:],
                                        lhsT=w_sb[:, dt_, 2 * H:2 * H + P],
                                        rhs=xt[:, dt_, n * QC:(n + 1) * QC],
                                        start=(dt_ == 0),
                                        stop=(dt_ == D_TILES - 1))
                                nc.scalar.copy(out=khT[:, col:col + QC], in_=ps[:])
                            else:
                                ps = pppool.tile([H, QC], F32, tag="pp")
                                for dt_ in range(D_TILES):
                                    nc.tensor.matmul(
                                        ps[:],
                                        lhsT=w_sb[:, dt_, 4 * H:5 * H],
                                        rhs=xt[:, dt_, n * QC:(n + 1) * QC],
                                        start=(dt_ == 0),
                                        stop=(dt_ == D_TILES - 1))
                                vtmp = vtpool.tile([H + 1, QC], F32, tag="vtmp")
                                nc.scalar.copy(out=vtmp[0:H, :], in_=ps[:])
                                nc.vector.memset(vtmp[H:H + 1, :], 1.0)
                                for tt in range(QC // P):
                                    t_glob = col // P + tt
                                    tp = pppool.tile([P, H + 1], F32, tag="pp")
                                    nc.tensor.transpose(
                                        tp[:], vtmp[:, tt * P:(tt + 1) * P],
                                        ident[0:H + 1, 0:H + 1])
                                    nc.scalar.copy(
                                        out=vh1[:, t_glob, :], in_=tp[:])

                    # attention tiles for the k-tiles this chunk covers
                    for t in range(ch * (XCW // P), (ch + 1) * (XCW // P)):
                        if t >= kt_lim:
                            continue
                        live = [s for s in range(N_SLOTS) if t < extents[s]]
                        sts = {}
                        for s in live:
                            sp = spool.tile([P, QC], F32, tag="S")
                            nc.tensor.matmul(
                                sp[:],
                                lhsT=khT[:, t * P:(t + 1) * P],
                                rhs=qhT[:, s * QC:(s + 1) * QC],
                                start=True, stop=True)
                            if need[s][t]:
                                m = mpool.tile([P, QC], F32, tag="m")
                                nc.sync.dma_start(out=m[:], in_=maskp[mask_idx])
                                mask_idx += 1
                                nc.vector.tensor_add(sp[:], sp[:], m[:])
                            sts[s] = sp
                        for s in live:
                            p = ppool.tile([P, QC], F32R, tag="P")
                            nc.scalar.activation(
                                out=p[:], in_=sts[s][:], func=Exp, scale=0.125)
                            nc.tensor.matmul(
                                oaccs[s][:],
                                lhsT=vh1[:, t, :],
                                rhs=p[:],
                                start=(t == 0), stop=(t == extents[s] - 1))

            # ---- phase 2: epilogue: transpose O' and divide by the sums ----
            with (
                tc.tile_pool(name="osb", bufs=2) as osbpool,
                tc.tile_pool(name="ot", bufs=2, space="PSUM") as otpool,
                tc.tile_pool(name="rec", bufs=2) as recpool,
                tc.tile_pool(name="ob", bufs=2) as obpool,
            ):
                for s in range(N_SLOTS):
                    osb = osbpool.tile([H + 1, QC], F32, tag="osb")
                    nc.scalar.copy(out=osb[:], in_=oaccs[s][:])
                    for j in range(QC // P):
                        ot = otpool.tile([P, H + 1], F32, tag="ot")
                        nc.tensor.transpose(
                            ot[:], osb[:, j * P:(j + 1) * P],
                            ident[0:H + 1, 0:H + 1])
                        rec = recpool.tile([P, 1], F32, tag="rec")
                        nc.vector.reciprocal(rec[:], ot[:, H:H + 1])
                        ob = obpool.tile([P, H], F32, tag="ob")
                        nc.vector.tensor_scalar_mul(ob[:], ot[:, 0:H], rec[:])
                        r0 = s * QC + j * P
                        nc.sync.dma_start(out=out[r0:r0 + P, :], in_=ob[:])

    nc.compile()
    return nc


def _get_nc(extents, need):
    key = (extents, need)
    if key not in _CACHE:
        _CACHE[key] = _build(extents, need)
    return _CACHE[key]


def _pack_w(Wq, Wk, Wv):
    z = np.zeros((D, H), np.float32)
    return np.ascontiguousarray(np.concatenate(
        [np.asarray(Wq, np.float32), z, np.asarray(Wk, np.float32), z,
         np.asarray(Wv, np.float32)], axis=1))


def _make_in_maps(q, k, v, wcat, mask, extents, need):
    order = _mask_order(extents, need)
    kTb = [np.ascontiguousarray(k[b].T) for b in range(B)]
    vTb = [np.ascontiguousarray(v[b].T) for b in range(B)]
    qTb = [np.ascontiguousarray(q[b].T) for b in range(B)]
    in_maps = []
    for c in range(N_CORES):
        b, par = divmod(c, 2)
        chunks = [2 * s + par for s in range(N_SLOTS)]
        qT_core = np.ascontiguousarray(np.concatenate(
            [qTb[b][:, g * QC:(g + 1) * QC] for g in chunks], axis=1))
        if order:
            mp = np.stack([
                np.where(
                    mask[chunks[s] * QC:(chunks[s] + 1) * QC,
                         t * P:(t + 1) * P].T, np.float32(0.0),
                    np.float32(NEG))
                for (s, t) in order]).astype(np.float32)
        else:
            mp = np.zeros((1, P, QC), np.float32)
        in_maps.append({
            "qT": qT_core, "kT": kTb[b], "vT": vTb[b],
            "w": wcat, "maskp": mp,
        })
    return in_maps


def _gather_out(results):
    outp = np.empty((B, T, H), np.float32)
    for c in range(N_CORES):
        b, par = divmod(c, 2)
        oc = results[c]["out"]
        for s in range(N_SLOTS):
            g = 2 * s + par
            outp[b, g * QC:(g + 1) * QC, :] = oc[s * QC:(s + 1) * QC, :]
    return outp


def kernel(q, k, v, Wq, Wk, Wv, attn_mask):
    global LAST_RESULT
    q = np.asarray(q, dtype=np.float32)
    k = np.asarray(k, dtype=np.float32)
    v = np.asarray(v, dtype=np.float32)
    mask = np.asarray(attn_mask).astype(bool)
    wcat = _pack_w(Wq, Wk, Wv)

    extents, need = _mask_schedule(mask)
    nc = _get_nc(extents, need)
    in_maps = _make_in_maps(q, k, v, wcat, mask, extents, need)

    res = run_bass_kernel_spmd(
        nc, in_maps, core_ids=list(range(N_CORES)),
        trace=bool(os.environ.get("KBENCH_TRACE")))
    LAST_RESULT = res
    return _gather_out(res.results)

